# revision 18
# baseline (speedup 1.0000x reference)
"""Trainium2 Bass kernel for nn_Attention (dense transformer block).

Reference computation per batch image (B=8, H=W=64, C=192, D=24, L=4096):
    q = x @ w_q; k = x @ w_k; v = x @ w_v          # [L, D]
    s = q @ k^T                                    # [L, L]
    beta = softmax(s, axis=-1)
    out = gamma * (beta @ v) @ w_o + x             # [L, C]

Sharding: pure data parallel, one image per NeuronCore (8 cores).

Per-core dataflow (matmuls bf16, fp32 PSUM accumulate). The PE array is
packed 4x both ways since the head dim (24) wastes the 128x128 array:
  - x^T arrives pre-transposed (bf16) from the host (pure layout transform).
  - all 16 q/k projections run in the prologue against group-stacked
    weights [C, 128]; PSUM pairs are copied out 1024 cols at a time.
  - scores are row-tiled: 4 key chunks concurrent in row groups 32g; each
    row group's output goes to its own PSUM bank (HW rule for row tiling).
  - softmax exp is split across ScalarE (exact, ACTIVATE) and VectorE
    (Schraudolph bf16 bit-trick: one fused mult+add with int16 convert).
  - attention accumulation is col-tiled: vhat chunks (M=32: v | ones |
    0pad) accumulate into group 32g of the stacked partials [128, W].
  - v is never projected separately: w_v rides in spare zero columns of
    wk_stack; v^T strips take a DRAM round trip through the xbar DMA
    transpose to become token-major vhat (zero PE cost).
  - epilogue: un = partials_T @ wo_stack [128, 193] (group-replicated
    gamma*w_o; col 192 collects the 4 softmax-denominator rows), then
    out = (un[:, :192] * (1/un[:, 192])) + x in ONE fused DVE
    scalar_tensor_tensor; epilogues are deferred one i-window.

HAM clock-gate management: PE_HAM throttles the PE to 1.2 GHz (matmul
cost (219+N)/1.2 instead of ~N/2.4 ns) after any ~3.4us activity window
it judges idle, and once dropped mid-kernel it effectively never
re-warms (re-warming needs a ~3.4us *fully busy* window, which a
dependency-limited pipeline never produces). Countermeasures:
  - DMAs issue first, split across both HWDGE rings; a long dense
    warm-up (FD=512) outlasts them; projections then run back-to-back
    with every dependency pre-satisfied; a dep-free filler burst bridges
    into the main loop.
  - steady state: cheap filler matmuls ([1,1]x[1,512] zero-adds into the
    armed partials bank -- exact no-ops touching only partition 0) are
    interleaved at the known stall points so no HAM window reads idle.

Softmax w/o max subtraction is safe: scores range ~[-50, 54];
exp(54) ~ 2e23 << fp32/bf16 max; row sums < 1e27.
"""

import numpy as np

import concourse.bass as bass
import concourse.tile as tile
from concourse import bacc, mybir
from concourse.bass_utils import run_bass_kernel_spmd
from concourse.masks import make_identity

F32 = mybir.dt.float32
BF16 = mybir.dt.bfloat16

B = 8
L = 4096          # tokens per image (64*64)
C = 192           # channels
D = 24            # head dim (q/k/v)
G = 4             # PE array packing groups
NCH = L // 128    # 32 chunks of 128 tokens
W = 512           # i-window (moving free dim per matmul)
NIW = L // W      # 8 i-windows
WIN = 512         # projection window (rhs free dim)
NWIN = L // WIN   # 8 windows
NQ = NCH // G     # 8 quads of key chunks
VW = 32           # padded vhat chunk width (v | ones | zeros)

N_WARMUP = 12            # FD=512 dense warm-up matmuls (covers HAM ramp + DMAs)


def filler_schedule(iw, t):
    """(pre_scores, pre_attn) filler counts per quad.

    Steady state: 1+1 (~0.43us of full-width PE streaming per ~1.3us quad
    keeps every HAM window busy).  Window 0 pipeline-fills with long exp
    waits, but most of that slack is absorbed by the chased projection
    pairs; a small extra burst covers the rest.
    """
    if iw == 0:
        return {0: (0, 2), 1: (1, 2)}.get(t, (1, 1))
    return (1, 1)


def build_graph():
    """Build the single-core Bass graph (SPMD: identical on all 8 cores)."""
    nc = bacc.Bacc(
        "TRN2", target_bir_lowering=False, debug=False, num_devices=8,
        name="attn_dp",
    )

    x_ext = nc.dram_tensor("x", [L, C], F32, kind="ExternalInput").ap()
    xt_ext = nc.dram_tensor("xt", [C, L], BF16, kind="ExternalInput").ap()
    # group-stacked projection weights [C, 128]: col 32g+d = w[:, d]
    wqs_ext = nc.dram_tensor("wq_stack", [C, 128], F32,
                             kind="ExternalInput").ap()
    wks_ext = nc.dram_tensor("wk_stack", [C, 128], F32,
                             kind="ExternalInput").ap()
    # wo_stack [128, 193]: rows 32g+d = gamma * w_o[d]; rows 32g+24 col 192 = 1
    wos_ext = nc.dram_tensor("wo_stack", [128, C + 1], F32,
                             kind="ExternalInput").ap()
    out_ext = nc.dram_tensor("out", [L, C], F32, kind="ExternalOutput").ap()

    with tile.TileContext(nc) as tc:
        _build(tc, x_ext, xt_ext, wqs_ext, wks_ext, wos_ext, out_ext)

    nc.compile()
    return nc


def _build(tc, x_ext, xt_ext, wqs_ext, wks_ext, wos_ext, out_ext):
    nc = tc.nc

    with (
        # ---- persistent SBUF ----
        tc.tile_pool(name="const", bufs=1) as const_pool,
        tc.tile_pool(name="xsb", bufs=1) as x_pool,
        tc.tile_pool(name="xT", bufs=1) as xT_pool,
        tc.tile_pool(name="qkT", bufs=1) as qkT_pool,
        tc.tile_pool(name="vhat", bufs=1) as vhat_pool,
        tc.tile_pool(name="expS", bufs=6) as expS_pool,
        tc.tile_pool(name="pt", bufs=2) as pt_pool,
        tc.tile_pool(name="outst", bufs=6) as outst_pool,
        tc.tile_pool(name="rden", bufs=6) as r_pool,
        # ---- PSUM (8 banks): scores/proj/epilogue 3x2 + partials 2x1 ----
        tc.tile_pool(name="ps_s", bufs=3, space="PSUM") as ps_scores,
        tc.tile_pool(name="ps_acc", bufs=2, space="PSUM") as ps_partials,
    ):
        # ================= DMAs first (overlap the PE warm-up) ==========
        # two big xt DMAs on the sync (SP) ring; weights on the scalar
        # (Act) ring: both rings issue in parallel, everything lands well
        # inside the warm-up.
        wstage = const_pool.tile([128, 760], F32)
        SR = 128
        st_wqsa = wstage[:, 0:SR]
        st_wqsb = wstage[:64, SR:2 * SR]
        st_wksa = wstage[:, 2 * SR:3 * SR]
        st_wksb = wstage[:64, 3 * SR:4 * SR]
        st_wos = wstage[:, 4 * SR:4 * SR + C + 1]
        nc.sync.dma_start(st_wksa, wks_ext[0:128, :])
        nc.sync.dma_start(st_wksb, wks_ext[128:192, :])
        nc.sync.dma_start(st_wqsa, wqs_ext[0:128, :])
        nc.sync.dma_start(st_wqsb, wqs_ext[128:192, :])
        nc.sync.dma_start(st_wos, wos_ext)

        xTa = xT_pool.tile([128, L], BF16)            # x^T rows 0..127
        xTb = xT_pool.tile([64, L], BF16)             # x^T rows 128..191
        nc.sync.dma_start(xTa[:], xt_ext[0:128, :])
        nc.sync.dma_start(xTb[:], xt_ext[128:192, :])

        # ================= constants (cheap engines) =================
        # warm-up dependencies first: the gpsimd queue runs these serially
        # and the first warm-up matmul gates on them
        ident = const_pool.tile([128, 128], BF16)
        make_identity(nc, ident[:])
        warm_rhs = const_pool.tile([128, W], BF16)
        nc.gpsimd.memset(warm_rhs[:], 0.0)

        # zeros for partials-bank arming + HAM filler matmuls.  The filler
        # stationary must be FULL-WIDTH: the HAM activity monitor does not
        # register narrow ([1,1]-stationary) matmuls as PE activity.
        zl = const_pool.tile([1, 128], BF16)
        zr = const_pool.tile([1, W], BF16)
        zlD = const_pool.tile([128, 128], BF16)
        nc.gpsimd.memset(zl[:], 0.0)
        nc.gpsimd.memset(zr[:], 0.0)
        nc.gpsimd.memset(zlD[:], 0.0)

        ones8 = const_pool.tile([8, L], BF16)
        nc.gpsimd.memset(ones8[:], 0.0)
        nc.gpsimd.memset(ones8[0:1, :], 1.0)

        # preload the exp_and_others ACT table set (~2.7us) during the
        # prologue so the first real exp doesn't stall the loop
        escr = const_pool.tile([1, 2], F32)
        nc.scalar.activation(escr[:], zl[0:1, 0:2],
                             mybir.ActivationFunctionType.Exp)

        # convert weights to bf16 on the Vector queue (the Scalar queue is
        # busy issuing the weight DMAs -- keeping the copies off it means
        # they chase the DMAs immediately)
        wksa = const_pool.tile([128, SR], BF16, tag="wksa")
        wksb = const_pool.tile([64, SR], BF16, tag="wksb")
        wqsa = const_pool.tile([128, SR], BF16, tag="wqsa")
        wqsb = const_pool.tile([64, SR], BF16, tag="wqsb")
        wos = const_pool.tile([128, C + 1], BF16, tag="wos")
        nc.vector.tensor_copy(wksa[:], st_wksa)
        nc.vector.tensor_copy(wksb[:], st_wksb)
        nc.vector.tensor_copy(wqsa[:], st_wqsa)
        nc.vector.tensor_copy(wqsb[:], st_wqsb)
        nc.vector.tensor_copy(wos[:], st_wos)

        # ================= PE warm-up =================
        # dense back-to-back FD=512 matmuls with no deps: un-throttles the
        # HAM clock gate (1.2 -> 2.4 GHz) and outlasts every prologue DMA,
        # so the projections start warm with all deps met.
        warm_ps = ps_scores.tile([128, W], F32, tag="s", name="warm_ps")
        for _ in range(N_WARMUP):
            nc.tensor.matmul(warm_ps[:], ident[:], warm_rhs[:],
                             start=True, stop=True)

        # ================= persistent tiles =================
        x_sb = x_pool.tile([128, NCH * C], F32)       # chunk c at cols [C*c,)
        kTs = qkT_pool.tile([128, L], BF16)           # stacked k^T replicas
        qTs = qkT_pool.tile([128, L], BF16)           # stacked q^T replicas
        vhat = vhat_pool.tile([128, NCH * VW], BF16)  # v | ones | zero pad
        vt = nc.dram_tensor("vt_scratch", [VW, L], BF16).ap()

        # ======== chased projections ========
        # window pair p: one [128,1024] PSUM tile = proj(2p) | proj(2p+1);
        # one 1024-col copy out, alternating ScalarE / VectorE.  Only pair 0
        # of k and q (plus vhat pair 0) runs in the prologue; the remaining
        # pairs are emitted inside the early windows, just ahead of the
        # quads that consume them -- real work that replaces fill-phase
        # fillers and cuts ~10us off the prologue span.
        def project_pair(dst, wa, wb, p, nm):
            ps = ps_scores.tile([128, 1024], F32, tag="s", name=f"pj{nm}{p}")
            for h in range(2):
                w = 2 * p + h
                sl = slice(WIN * w, WIN * (w + 1))
                psl = slice(512 * h, 512 * (h + 1))
                nc.tensor.matmul(ps[:, psl], wa[:], xTa[:, sl],
                                 start=True, stop=False)
                nc.tensor.matmul(ps[:, psl], wb[:], xTb[:, sl],
                                 start=False, stop=True)
            dsl = slice(1024 * p, 1024 * (p + 1))
            if p % 2 == 0:
                nc.scalar.copy(dst[:, dsl], ps[:])
            else:
                nc.vector.tensor_copy(dst[:, dsl], ps[:])

        vhat_view = vhat.rearrange("p (j d) -> p j d", d=VW)

        def emit_vt_pair(p):
            # v^T strips for token pair p out of kTs -> DRAM, then
            # xbar-transpose back as token-major vhat chunks 8p..8p+7
            csl = slice(1024 * p, 1024 * (p + 1))
            nc.sync.dma_start(vt[0:8, csl], kTs[24:32, csl])
            nc.sync.dma_start(vt[8:16, csl], kTs[56:64, csl])
            nc.sync.dma_start(vt[16:24, csl], kTs[88:96, csl])
            nc.sync.dma_start(vt[24:32, csl], ones8[:, csl])
            nc.sync.dma_start_transpose(out=vhat_view[:, 8 * p:8 * (p + 1), :],
                                        in_=vt[:, csl])

        project_pair(kTs, wksa, wksb, 0, "k")
        emit_vt_pair(0)
        project_pair(qTs, wqsa, wqsb, 0, "q")

        pt_tiles = {}
        ot_tiles = {}
        # chunk-major DRAM views: rows (s p) -> [p, s, c] so one DMA moves a
        # whole window's 4 chunks between DRAM and the chunked SBUF layout
        oview = out_ext.rearrange("(s p) c -> p s c", p=128)
        xview = x_ext.rearrange("(s p) c -> p s c", p=128)
        x_sbv = x_sb.rearrange("p (s c) -> p s c", c=C)

        def emit_epilogue(piw, s):
            ptb = pt_tiles[piw]
            cidx = (W // 128) * piw + s  # 128-token chunk index
            if s == 0:
                ot_tiles[piw] = outst_pool.tile([128, 4 * C], F32,
                                                name=f"ot{piw}", tag="ot")
            ot = ot_tiles[piw]
            ep = ps_scores.tile([128, C + 1], F32, tag="s", name=f"ep{piw}_{s}")
            nc.tensor.matmul(ep[:], ptb[:, 128 * s:128 * (s + 1)],
                             wos[:], start=True, stop=True)
            rr = r_pool.tile([128, 1], F32, name=f"rr{piw}_{s}", tag="rr")
            nc.vector.reciprocal(rr[:], ep[:, C:C + 1])
            # out = (un * (1/denom)) + x in one fused DVE op
            nc.vector.scalar_tensor_tensor(
                ot[:, C * s:C * (s + 1)], ep[:, 0:C], rr[:],
                x_sb[:, C * cidx:C * (cidx + 1)],
                op0=mybir.AluOpType.mult, op1=mybir.AluOpType.add)
            if s == W // 128 - 1:
                otv = ot.rearrange("p (s c) -> p s c", c=C)
                nc.sync.dma_start(oview[:, 4 * piw:4 * piw + 4, :], otv)

        # ================= main loop =================
        def emit_attn(partials_, t, ess):
            for g in (2, 3, 0, 1):
                j = G * t + g
                nc.tensor.matmul(
                    partials_[32 * g:32 * g + VW, :],
                    vhat[:, VW * j:VW * (j + 1)],
                    ess[g // 2][:, 512 * (g % 2):512 * (g % 2 + 1)],
                    start=False, stop=(t == NQ - 1),
                    tile_position=(0, 32 * g),
                    skip_group_check=True,
                )

        def emit_fillers(partials_, n):
            # full-width [128,128]x[128,512] zero-add into the armed partials
            # bank: numerically exact no-op, ~215ns of dense PE streaming each
            # that the HAM activity monitor actually registers.
            for _ in range(n):
                nc.tensor.matmul(partials_[:, :], zlD[:], warm_rhs[:],
                                 start=False, stop=False,
                                 skip_group_check=True)

        pending = None
        pending_attn = None
        for iw in range(NIW):
            isl = slice(W * iw, W * (iw + 1))
            # residual x chunks for this window's epilogue (runs during iw+1)
            nc.sync.dma_start(x_sbv[:, 4 * iw:4 * iw + 4, :],
                              xview[:, 4 * iw:4 * iw + 4, :])
            partials = ps_partials.tile([128, W], F32, name=f"partials{iw}",
                                        tag="acc")
            # zero-init the bank and set has_written on all 128 partitions so
            # the col-tiled accumulating matmuls below can all use start=False
            nc.tensor.matmul(partials[:, :], zl[:], zr[:],
                             start=True, stop=False, skip_group_check=True)
            for t in range(NQ):
                qidx = NQ * iw + t
                n_pre, n_post = filler_schedule(iw, t)
                emit_fillers(partials, n_pre)
                scs = [ps_scores.tile([128, 1024], F32, tag="s",
                                      name=f"sc{iw}_{t}_0"),
                       ps_scores.tile([128, 1024], F32, tag="s",
                                      name=f"sc{iw}_{t}_1")]
                for g in range(G):
                    j = G * t + g
                    nc.tensor.matmul(
                        scs[g // 2][:, 512 * (g % 2):512 * (g % 2 + 1)],
                        kTs[32 * g:32 * g + 32, 128 * j:128 * (j + 1)],
                        qTs[32 * g:32 * g + 32, isl],
                        start=True, stop=True,
                        tile_position=(32 * g, 0),
                    )
                # chase the remaining projection pairs just ahead of their
                # consumers: k-pair/vhat-pair p before scores quad t=2p of
                # window 0; q-pair p two windows ahead of window 2p
                if iw == 0 and t in (0, 2, 4):
                    project_pair(kTs, wksa, wksb, t // 2 + 1, "k")
                    emit_vt_pair(t // 2 + 1)
                if (iw, t) in ((0, 6), (2, 0), (4, 0)):
                    project_pair(qTs, wqsa, wqsb, iw // 2 + 1, "q")
                # exp split: ScalarE always takes h=0; DVE takes h=1 except
                # every 8th quad (ratio ~72/56 balances total engine load)
                both_sc = (qidx % 8) == 0
                ess = []
                for h in range(2):
                    es = expS_pool.tile([128, 1024], BF16,
                                        name=f"es{iw}_{t}_{h}", tag="es")
                    if h == 1 and not both_sc:
                        # Schraudolph exp on DVE: bf16 bits = round(s*log2e*128
                        # + 127*128); one fused mult+add with int16 convert
                        nc.vector.tensor_scalar(
                            es[:].bitcast(mybir.dt.int16), scs[h][:],
                            184.66496580927026, 16256.0,
                            op0=mybir.AluOpType.mult, op1=mybir.AluOpType.add)
                    else:
                        nc.scalar.activation(es[:], scs[h][:],
                                             mybir.ActivationFunctionType.Exp)
                    ess.append(es)
                if pending is not None and t < W // 128:
                    emit_epilogue(pending, t)
                emit_fillers(partials, n_post)
                # attention emission is deferred one quad: the PE fills the
                # exp wait with the next quad's scores instead of idling
                if pending_attn is not None:
                    emit_attn(*pending_attn)
                pending_attn = (partials, t, ess)

            emit_attn(*pending_attn)
            pending_attn = None
            ptb = pt_pool.tile([128, W], BF16, name=f"ptb{iw}", tag="ptb")
            nc.scalar.copy(ptb[:], partials[:])
            pt_tiles[iw] = ptb
            pending = iw
        for s in range(W // 128):
            emit_epilogue(pending, s)


_CACHE = {}


def _get_graph():
    if "nc" not in _CACHE:
        _CACHE["nc"] = build_graph()
    return _CACHE["nc"]


def make_in_maps(tensor, w_q, w_k, w_v, w_o, gamma):
    import ml_dtypes
    x = np.ascontiguousarray(np.asarray(tensor, dtype=np.float32)).reshape(B, L, C)
    xt = np.ascontiguousarray(
        x.transpose(0, 2, 1).astype(ml_dtypes.bfloat16))  # [B, C, L] bf16
    wq = np.asarray(w_q, dtype=np.float32)
    wk = np.asarray(w_k, dtype=np.float32)
    wv = np.ascontiguousarray(np.asarray(w_v, dtype=np.float32))
    wo = np.asarray(w_o, dtype=np.float32)

    wq_stack = np.zeros((C, 128), dtype=np.float32)
    wk_stack = np.zeros((C, 128), dtype=np.float32)
    for g in range(G):
        wq_stack[:, 32 * g:32 * g + D] = wq
        wk_stack[:, 32 * g:32 * g + D] = wk
    # w_v rides in the spare zero columns of wk_stack (contracts against
    # zero rows of the q stack, so scores are unaffected); the k-projection
    # then produces v^T rows for free.
    wk_stack[:, 24:32] = wv[:, 0:8]
    wk_stack[:, 56:64] = wv[:, 8:16]
    wk_stack[:, 88:96] = wv[:, 16:24]

    wo_stack = np.zeros((128, C + 1), dtype=np.float32)
    for g in range(G):
        wo_stack[32 * g:32 * g + D, :C] = wo * np.float32(gamma)
        wo_stack[32 * g + D, C] = 1.0

    return [
        {"x": np.ascontiguousarray(x[b]), "xt": xt[b], "wq_stack": wq_stack,
         "wk_stack": wk_stack, "wo_stack": wo_stack}
        for b in range(B)
    ]


def kernel(tensor, w_q, w_k, w_v, w_o, gamma):
    nc = _get_graph()
    in_maps = make_in_maps(tensor, w_q, w_k, w_v, w_o, gamma)
    res = run_bass_kernel_spmd(nc, in_maps, core_ids=list(range(B)))
    out = np.stack([np.asarray(res.results[b]["out"]) for b in range(B)])
    return out.reshape(B, 64, 64, C).astype(np.float32)


# revision 25
# speedup vs baseline: 1.1758x; 1.1758x over previous
"""Trainium2 Bass kernel for nn_Attention (dense transformer block).

Reference computation per batch image (B=8, H=W=64, C=192, D=24, L=4096):
    q = x @ w_q; k = x @ w_k; v = x @ w_v          # [L, D]
    s = q @ k^T                                    # [L, L]
    beta = softmax(s, axis=-1)
    out = gamma * (beta @ v) @ w_o + x             # [L, C]

Sharding: pure data parallel, one image per NeuronCore (8 cores).

Per-core dataflow (matmuls bf16, fp32 PSUM accumulate). The PE array is
packed 4x both ways since the head dim (24) wastes the 128x128 array:
  - x^T arrives pre-transposed (bf16) from the host (pure layout transform).
  - all 16 q/k projections run in the prologue against group-stacked
    weights [C, 128]; PSUM pairs are copied out 1024 cols at a time.
  - scores are row-tiled: 4 key chunks concurrent in row groups 32g; each
    row group's output goes to its own PSUM bank (HW rule for row tiling).
  - softmax exp is split across ScalarE (exact, ACTIVATE) and VectorE
    (Schraudolph bf16 bit-trick: one fused mult+add with int16 convert).
  - attention accumulation is col-tiled: vhat chunks (M=32: v | ones |
    0pad) accumulate into group 32g of the stacked partials [128, W].
  - v is never projected separately: w_v rides in spare zero columns of
    wk_stack; v^T strips take a DRAM round trip through the xbar DMA
    transpose to become token-major vhat (zero PE cost).
  - epilogue: un = partials_T @ wo_stack [128, 193] (group-replicated
    gamma*w_o; col 192 collects the 4 softmax-denominator rows), then
    out = (un[:, :192] * (1/un[:, 192])) + x in ONE fused DVE
    scalar_tensor_tensor; epilogues are deferred one i-window.

HAM clock-gate management: PE_HAM throttles the PE to 1.2 GHz (matmul
cost (219+N)/1.2 instead of ~N/2.4 ns) after any ~3.4us activity window
it judges idle, and once dropped mid-kernel it effectively never
re-warms (re-warming needs a ~3.4us *fully busy* window, which a
dependency-limited pipeline never produces). Countermeasures:
  - DMAs issue first, split across both HWDGE rings; a long dense
    warm-up (FD=512) outlasts them; projections then run back-to-back
    with every dependency pre-satisfied; a dep-free filler burst bridges
    into the main loop.
  - steady state: cheap filler matmuls ([1,1]x[1,512] zero-adds into the
    armed partials bank -- exact no-ops touching only partition 0) are
    interleaved at the known stall points so no HAM window reads idle.

Softmax w/o max subtraction is safe: scores range ~[-50, 54];
exp(54) ~ 2e23 << fp32/bf16 max; row sums < 1e27.
"""

import numpy as np

import concourse.bass as bass
import concourse.tile as tile
from concourse import bacc, mybir
from concourse.bass_utils import run_bass_kernel_spmd
from concourse.masks import make_identity

F32 = mybir.dt.float32
BF16 = mybir.dt.bfloat16

B = 8
L = 4096          # tokens per image (64*64)
C = 192           # channels
D = 24            # head dim (q/k/v)
G = 4             # PE array packing groups
NCH = L // 128    # 32 chunks of 128 tokens
W = 512           # i-window (moving free dim per matmul)
NIW = L // W      # 8 i-windows
WIN = 512         # projection window (rhs free dim)
NWIN = L // WIN   # 8 windows
NQ = NCH // G     # 8 quads of key chunks
VW = 32           # padded vhat chunk width (v | ones | zeros)

N_WARMUP = 12            # FD=512 dense warm-up matmuls (covers HAM ramp + DMAs)


def filler_schedule(iw, t):
    """(pre_scores, pre_attn) filler counts per quad.

    Steady state: 1+1 (~0.43us of full-width PE streaming per ~1.3us quad
    keeps every HAM window busy).  Window 0 pipeline-fills with long exp
    waits, but most of that slack is absorbed by the chased projection
    pairs; a small extra burst covers the rest.
    """
    if iw == 0:
        return {0: (0, 2), 1: (1, 2)}.get(t, (1, 1))
    return (1, 1)


def build_graph():
    """Build the single-core Bass graph (SPMD: identical on all 8 cores)."""
    nc = bacc.Bacc(
        "TRN2", target_bir_lowering=False, debug=False, num_devices=8,
        name="attn_dp",
    )

    x_ext = nc.dram_tensor("x", [L, C], F32, kind="ExternalInput").ap()
    xt_ext = nc.dram_tensor("xt", [C, L], BF16, kind="ExternalInput").ap()
    # group-stacked projection weights [C, 128]: col 32g+d = w[:, d]
    wqs_ext = nc.dram_tensor("wq_stack", [C, 128], F32,
                             kind="ExternalInput").ap()
    wks_ext = nc.dram_tensor("wk_stack", [C, 128], F32,
                             kind="ExternalInput").ap()
    # wo_stack [128, 193]: rows 32g+d = gamma * w_o[d]; rows 32g+24 col 192 = 1
    wos_ext = nc.dram_tensor("wo_stack", [128, C + 1], F32,
                             kind="ExternalInput").ap()
    out_ext = nc.dram_tensor("out", [L, C], F32, kind="ExternalOutput").ap()

    with tile.TileContext(nc) as tc:
        _build(tc, x_ext, xt_ext, wqs_ext, wks_ext, wos_ext, out_ext)

    nc.compile()
    return nc


def _build(tc, x_ext, xt_ext, wqs_ext, wks_ext, wos_ext, out_ext):
    nc = tc.nc

    with (
        # ---- persistent SBUF ----
        tc.tile_pool(name="const", bufs=1) as const_pool,
        tc.tile_pool(name="xsb", bufs=1) as x_pool,
        tc.tile_pool(name="xT", bufs=1) as xT_pool,
        tc.tile_pool(name="qkT", bufs=1) as qkT_pool,
        tc.tile_pool(name="vhat", bufs=1) as vhat_pool,
        tc.tile_pool(name="expS", bufs=6) as expS_pool,
        tc.tile_pool(name="pt", bufs=3) as pt_pool,
        tc.tile_pool(name="outst", bufs=3) as outst_pool,
        tc.tile_pool(name="rden", bufs=6) as r_pool,
        # ---- PSUM (8 banks): scores/proj/epilogue 3x2 + partials 2x1 ----
        tc.tile_pool(name="ps_s", bufs=3, space="PSUM") as ps_scores,
        tc.tile_pool(name="ps_acc", bufs=2, space="PSUM") as ps_partials,
    ):
        # ================= DMAs first (overlap the PE warm-up) ==========
        # two big xt DMAs on the sync (SP) ring; weights on the scalar
        # (Act) ring: both rings issue in parallel, everything lands well
        # inside the warm-up.
        wstage = const_pool.tile([128, 760], F32)
        SR = 128
        st_wqsa = wstage[:, 0:SR]
        st_wqsb = wstage[:64, SR:2 * SR]
        st_wksa = wstage[:, 2 * SR:3 * SR]
        st_wksb = wstage[:64, 3 * SR:4 * SR]
        st_wos = wstage[:, 4 * SR:4 * SR + C + 1]
        nc.sync.dma_start(st_wksa, wks_ext[0:128, :])
        nc.sync.dma_start(st_wksb, wks_ext[128:192, :])
        nc.sync.dma_start(st_wqsa, wqs_ext[0:128, :])
        nc.sync.dma_start(st_wqsb, wqs_ext[128:192, :])
        nc.sync.dma_start(st_wos, wos_ext)

        xTa = xT_pool.tile([128, L], BF16)            # x^T rows 0..127
        xTb = xT_pool.tile([64, L], BF16)             # x^T rows 128..191
        nc.sync.dma_start(xTa[:], xt_ext[0:128, :])
        nc.sync.dma_start(xTb[:], xt_ext[128:192, :])

        # ================= constants (cheap engines) =================
        # warm-up dependencies first: the gpsimd queue runs these serially
        # and the first warm-up matmul gates on them
        ident = const_pool.tile([128, 128], BF16)
        make_identity(nc, ident[:])
        warm_rhs = const_pool.tile([128, W], BF16)
        nc.gpsimd.memset(warm_rhs[:], 0.0)

        # zeros for partials-bank arming + HAM filler matmuls.  The filler
        # stationary must be FULL-WIDTH: the HAM activity monitor does not
        # register narrow ([1,1]-stationary) matmuls as PE activity.
        zl = const_pool.tile([1, 128], BF16)
        zr = const_pool.tile([1, W], BF16)
        zlD = const_pool.tile([128, 128], BF16)
        nc.gpsimd.memset(zl[:], 0.0)
        nc.gpsimd.memset(zr[:], 0.0)
        nc.gpsimd.memset(zlD[:], 0.0)

        ones8 = const_pool.tile([8, L], BF16)
        nc.gpsimd.memset(ones8[:], 0.0)
        nc.gpsimd.memset(ones8[0:1, :], 1.0)

        # preload the exp_and_others ACT table set (~2.7us) during the
        # prologue so the first real exp doesn't stall the loop
        escr = const_pool.tile([1, 2], F32)
        nc.scalar.activation(escr[:], zl[0:1, 0:2],
                             mybir.ActivationFunctionType.Exp)

        # convert weights to bf16 on the Vector queue (the Scalar queue is
        # busy issuing the weight DMAs -- keeping the copies off it means
        # they chase the DMAs immediately)
        wksa = const_pool.tile([128, SR], BF16, tag="wksa")
        wksb = const_pool.tile([64, SR], BF16, tag="wksb")
        wqsa = const_pool.tile([128, SR], BF16, tag="wqsa")
        wqsb = const_pool.tile([64, SR], BF16, tag="wqsb")
        wos = const_pool.tile([128, C + 1], BF16, tag="wos")
        nc.vector.tensor_copy(wksa[:], st_wksa)
        nc.vector.tensor_copy(wksb[:], st_wksb)
        nc.vector.tensor_copy(wqsa[:], st_wqsa)
        nc.vector.tensor_copy(wqsb[:], st_wqsb)
        nc.vector.tensor_copy(wos[:], st_wos)

        # ================= PE warm-up =================
        # dense back-to-back FD=512 matmuls with no deps: un-throttles the
        # HAM clock gate (1.2 -> 2.4 GHz) and outlasts every prologue DMA,
        # so the projections start warm with all deps met.
        warm_ps = ps_scores.tile([128, W], F32, tag="s", name="warm_ps")
        for _ in range(N_WARMUP):
            nc.tensor.matmul(warm_ps[:], ident[:], warm_rhs[:],
                             start=True, stop=True)

        # ================= persistent tiles =================
        x_sb = x_pool.tile([128, NCH * C], F32)       # chunk c at cols [C*c,)
        kTs = qkT_pool.tile([128, L], BF16)           # stacked k^T replicas
        qTs = qkT_pool.tile([128, L], BF16)           # stacked q^T replicas
        vhat = vhat_pool.tile([128, NCH * VW], BF16)  # v | ones | zero pad
        vt = nc.dram_tensor("vt_scratch", [VW, L], BF16).ap()

        # ======== chased projections ========
        # window pair p: one [128,1024] PSUM tile = proj(2p) | proj(2p+1);
        # one 1024-col copy out, alternating ScalarE / VectorE.  Only pair 0
        # of k and q (plus vhat pair 0) runs in the prologue; the remaining
        # pairs are emitted inside the early windows, just ahead of the
        # quads that consume them -- real work that replaces fill-phase
        # fillers and cuts ~10us off the prologue span.
        def project_pair(dst, wa, wb, p, nm):
            ps = ps_scores.tile([128, 1024], F32, tag="s", name=f"pj{nm}{p}")
            for h in range(2):
                w = 2 * p + h
                sl = slice(WIN * w, WIN * (w + 1))
                psl = slice(512 * h, 512 * (h + 1))
                nc.tensor.matmul(ps[:, psl], wa[:], xTa[:, sl],
                                 start=True, stop=False)
                nc.tensor.matmul(ps[:, psl], wb[:], xTb[:, sl],
                                 start=False, stop=True)
            dsl = slice(1024 * p, 1024 * (p + 1))
            par = p if nm == "k" else p + 1   # k0 on Sc, q0 on Ve, ...
            if par % 2 == 0:
                nc.scalar.copy(dst[:, dsl], ps[:])
            else:
                nc.vector.tensor_copy(dst[:, dsl], ps[:])

        vhat_view = vhat.rearrange("p (j d) -> p j d", d=VW)

        def emit_vt_pair(p):
            # v^T strips for token pair p out of kTs -> DRAM, then
            # xbar-transpose back as token-major vhat chunks 8p..8p+7
            csl = slice(1024 * p, 1024 * (p + 1))
            nc.sync.dma_start(vt[0:8, csl], kTs[24:32, csl])
            nc.sync.dma_start(vt[8:16, csl], kTs[56:64, csl])
            nc.sync.dma_start(vt[16:24, csl], kTs[88:96, csl])
            nc.sync.dma_start(vt[24:32, csl], ones8[:, csl])
            nc.sync.dma_start_transpose(out=vhat_view[:, 8 * p:8 * (p + 1), :],
                                        in_=vt[:, csl])

        project_pair(kTs, wksa, wksb, 0, "k")
        emit_vt_pair(0)
        project_pair(qTs, wqsa, wqsb, 0, "q")

        pt_tiles = {}
        ot_tiles = {}
        # chunk-major DRAM views: rows (s p) -> [p, s, c] so one DMA moves a
        # whole window's 4 chunks between DRAM and the chunked SBUF layout
        oview = out_ext.rearrange("(s p) c -> p s c", p=128)
        xview = x_ext.rearrange("(s p) c -> p s c", p=128)
        x_sbv = x_sb.rearrange("p (s c) -> p s c", c=C)

        def emit_epilogue(piw, s):
            ptb = pt_tiles[piw]
            cidx = (W // 128) * piw + s  # 128-token chunk index
            if s == 0:
                ot_tiles[piw] = outst_pool.tile([128, 4 * C], F32,
                                                name=f"ot{piw}", tag="ot")
            ot = ot_tiles[piw]
            ep = ps_scores.tile([128, C + 1], F32, tag="s", name=f"ep{piw}_{s}")
            nc.tensor.matmul(ep[:], ptb[:, 128 * s:128 * (s + 1)],
                             wos[:], start=True, stop=True)
            rr = r_pool.tile([128, 1], F32, name=f"rr{piw}_{s}", tag="rr")
            nc.vector.reciprocal(rr[:], ep[:, C:C + 1])
            # out = (un * (1/denom)) + x in one fused DVE op
            nc.vector.scalar_tensor_tensor(
                ot[:, C * s:C * (s + 1)], ep[:, 0:C], rr[:],
                x_sb[:, C * cidx:C * (cidx + 1)],
                op0=mybir.AluOpType.mult, op1=mybir.AluOpType.add)
            if s == W // 128 - 1:
                otv = ot.rearrange("p (s c) -> p s c", c=C)
                nc.sync.dma_start(oview[:, 4 * piw:4 * piw + 4, :], otv)

        # ================= main loop =================
        def emit_attn(partials_, t, ess):
            for g in (2, 3, 0, 1):
                j = G * t + g
                nc.tensor.matmul(
                    partials_[32 * g:32 * g + VW, :],
                    vhat[:, VW * j:VW * (j + 1)],
                    ess[g // 2][:, 512 * (g % 2):512 * (g % 2 + 1)],
                    start=False, stop=(t == NQ - 1),
                    tile_position=(0, 32 * g),
                    skip_group_check=True,
                )

        def emit_fillers(partials_, n, rhs=None):
            # full-width [128,128]x[128,512] zero-add into the armed partials
            # bank: numerically exact no-op (zero stationary), ~215ns of dense
            # PE streaming each that the HAM activity monitor registers.  An
            # explicit rhs gives the filler a data dependency so the
            # scheduler places it at a specific stall point.
            for _ in range(n):
                nc.tensor.matmul(partials_[:, :], zlD[:],
                                 warm_rhs[:] if rhs is None else rhs,
                                 start=False, stop=False,
                                 skip_group_check=True)

        pending_attn = None
        for iw in range(NIW):
            isl = slice(W * iw, W * (iw + 1))
            # residual x chunks for this window's epilogue (runs during iw+2)
            nc.sync.dma_start(x_sbv[:, 4 * iw:4 * iw + 4, :],
                              xview[:, 4 * iw:4 * iw + 4, :])
            partials = ps_partials.tile([128, W], F32, name=f"partials{iw}",
                                        tag="acc")
            # zero-init the bank and set has_written on all 128 partitions so
            # the col-tiled accumulating matmuls below can all use start=False
            nc.tensor.matmul(partials[:, :], zl[:], zr[:],
                             start=True, stop=False, skip_group_check=True)
            if iw == 0:
                # bridge fillers gated on the q-pair0 copy: they occupy the
                # PE exactly across the prologue -> main-loop hand-off
                emit_fillers(partials, 2, rhs=qTs[:, 0:W])
            for t in range(NQ):
                qidx = NQ * iw + t
                n_pre, n_post = filler_schedule(iw, t)
                emit_fillers(partials, n_pre)
                scs = [ps_scores.tile([128, 1024], F32, tag="s",
                                      name=f"sc{iw}_{t}_0"),
                       ps_scores.tile([128, 1024], F32, tag="s",
                                      name=f"sc{iw}_{t}_1")]
                for g in range(G):
                    j = G * t + g
                    nc.tensor.matmul(
                        scs[g // 2][:, 512 * (g % 2):512 * (g % 2 + 1)],
                        kTs[32 * g:32 * g + 32, 128 * j:128 * (j + 1)],
                        qTs[32 * g:32 * g + 32, isl],
                        start=True, stop=True,
                        tile_position=(32 * g, 0),
                    )
                # chase the remaining projection pairs just ahead of their
                # consumers: k-pair/vhat-pair p before scores quad t=2p of
                # window 0; q-pair p two windows ahead of window 2p
                if iw == 0 and t in (0, 2, 4):
                    project_pair(kTs, wksa, wksb, t // 2 + 1, "k")
                    emit_vt_pair(t // 2 + 1)
                if (iw, t) in ((0, 6), (2, 0), (4, 0)):
                    project_pair(qTs, wqsa, wqsb, iw // 2 + 1, "q")
                # exp split: ScalarE takes h=0; DVE takes h=1, except every
                # 8th quad splits its h=1 tile across BOTH engines (net
                # ratio ~72/56 balances engine load without a 2-tile Sc
                # hiccup feeding queue jitter)
                split_q = (qidx % 8) == 0
                ess = []
                for h in range(2):
                    es = expS_pool.tile([128, 1024], BF16,
                                        name=f"es{iw}_{t}_{h}", tag="es")
                    if h == 0:
                        nc.scalar.activation(es[:], scs[h][:],
                                             mybir.ActivationFunctionType.Exp)
                    elif split_q:
                        nc.scalar.activation(es[:, 0:512], scs[h][:, 0:512],
                                             mybir.ActivationFunctionType.Exp)
                        nc.vector.tensor_scalar(
                            es[:, 512:1024].bitcast(mybir.dt.int16),
                            scs[h][:, 512:1024],
                            184.66496580927026, 16256.0,
                            op0=mybir.AluOpType.mult, op1=mybir.AluOpType.add)
                    else:
                        # Schraudolph exp on DVE: bf16 bits = round(s*log2e*128
                        # + 127*128); one fused mult+add with int16 convert
                        nc.vector.tensor_scalar(
                            es[:].bitcast(mybir.dt.int16), scs[h][:],
                            184.66496580927026, 16256.0,
                            op0=mybir.AluOpType.mult, op1=mybir.AluOpType.add)
                    ess.append(es)
                # epilogue deferred TWO windows: the ptb copy then has a full
                # window to drain through the exp backlog, so the in-order
                # PE queue never stalls on the epilogue matmul's ptb wait
                if iw >= 2 and t < W // 128:
                    emit_epilogue(iw - 2, t)
                if iw == NIW - 1 and t >= NQ - W // 128:
                    # squeeze the penultimate window's epilogues into the
                    # back half of the last window to shorten the tail
                    emit_epilogue(NIW - 2, t - (NQ - W // 128))
                emit_fillers(partials, n_post)
                # attention emission is deferred one quad: the PE fills the
                # exp wait with the next quad's scores instead of idling
                if pending_attn is not None:
                    emit_attn(*pending_attn)
                pending_attn = (partials, t, ess)

            emit_attn(*pending_attn)
            pending_attn = None
            ptb = pt_pool.tile([128, W], BF16, name=f"ptb{iw}", tag="ptb")
            nc.scalar.copy(ptb[:], partials[:])
            pt_tiles[iw] = ptb
        for piw in (NIW - 2, NIW - 1):
            for s in range(W // 128):
                emit_epilogue(piw, s)


_CACHE = {}


def _get_graph():
    if "nc" not in _CACHE:
        _CACHE["nc"] = build_graph()
    return _CACHE["nc"]


def make_in_maps(tensor, w_q, w_k, w_v, w_o, gamma):
    import ml_dtypes
    x = np.ascontiguousarray(np.asarray(tensor, dtype=np.float32)).reshape(B, L, C)
    xt = np.ascontiguousarray(
        x.transpose(0, 2, 1).astype(ml_dtypes.bfloat16))  # [B, C, L] bf16
    wq = np.asarray(w_q, dtype=np.float32)
    wk = np.asarray(w_k, dtype=np.float32)
    wv = np.ascontiguousarray(np.asarray(w_v, dtype=np.float32))
    wo = np.asarray(w_o, dtype=np.float32)

    wq_stack = np.zeros((C, 128), dtype=np.float32)
    wk_stack = np.zeros((C, 128), dtype=np.float32)
    for g in range(G):
        wq_stack[:, 32 * g:32 * g + D] = wq
        wk_stack[:, 32 * g:32 * g + D] = wk
    # w_v rides in the spare zero columns of wk_stack (contracts against
    # zero rows of the q stack, so scores are unaffected); the k-projection
    # then produces v^T rows for free.
    wk_stack[:, 24:32] = wv[:, 0:8]
    wk_stack[:, 56:64] = wv[:, 8:16]
    wk_stack[:, 88:96] = wv[:, 16:24]

    wo_stack = np.zeros((128, C + 1), dtype=np.float32)
    for g in range(G):
        wo_stack[32 * g:32 * g + D, :C] = wo * np.float32(gamma)
        wo_stack[32 * g + D, C] = 1.0

    return [
        {"x": np.ascontiguousarray(x[b]), "xt": xt[b], "wq_stack": wq_stack,
         "wk_stack": wk_stack, "wo_stack": wo_stack}
        for b in range(B)
    ]


def kernel(tensor, w_q, w_k, w_v, w_o, gamma):
    nc = _get_graph()
    in_maps = make_in_maps(tensor, w_q, w_k, w_v, w_o, gamma)
    res = run_bass_kernel_spmd(nc, in_maps, core_ids=list(range(B)))
    out = np.stack([np.asarray(res.results[b]["out"]) for b in range(B)])
    return out.reshape(B, 64, 64, C).astype(np.float32)


# revision 30
# speedup vs baseline: 1.1841x; 1.0071x over previous
"""Trainium2 Bass kernel for nn_Attention (dense transformer block).

Reference computation per batch image (B=8, H=W=64, C=192, D=24, L=4096):
    q = x @ w_q; k = x @ w_k; v = x @ w_v          # [L, D]
    s = q @ k^T                                    # [L, L]
    beta = softmax(s, axis=-1)
    out = gamma * (beta @ v) @ w_o + x             # [L, C]

Sharding: pure data parallel, one image per NeuronCore (8 cores).

Per-core dataflow (matmuls bf16, fp32 PSUM accumulate). The PE array is
packed 4x both ways since the head dim (24) wastes the 128x128 array:
  - x^T arrives pre-transposed (bf16) from the host (pure layout transform).
  - all 16 q/k projections run in the prologue against group-stacked
    weights [C, 128]; PSUM pairs are copied out 1024 cols at a time.
  - scores are row-tiled: 4 key chunks concurrent in row groups 32g; each
    row group's output goes to its own PSUM bank (HW rule for row tiling).
  - softmax exp is split across ScalarE (exact, ACTIVATE) and VectorE
    (Schraudolph bf16 bit-trick: one fused mult+add with int16 convert).
  - attention accumulation is col-tiled: vhat chunks (M=32: v | ones |
    0pad) accumulate into group 32g of the stacked partials [128, W].
  - v is never projected separately: w_v rides in spare zero columns of
    wk_stack; v^T strips take a DRAM round trip through the xbar DMA
    transpose to become token-major vhat (zero PE cost).
  - epilogue: un = partials_T @ wo_stack [128, 193] (group-replicated
    gamma*w_o; col 192 collects the 4 softmax-denominator rows), then
    out = (un[:, :192] * (1/un[:, 192])) + x in ONE fused DVE
    scalar_tensor_tensor; epilogues are deferred one i-window.

HAM clock-gate management: PE_HAM throttles the PE to 1.2 GHz (matmul
cost (219+N)/1.2 instead of ~180+N/2.4 ns) after any ~3.4us activity
window it judges idle (the threshold is strict: a window at 85% busy
with one ~700ns gap still drops it), and re-warming requires a ~3.4us
near-fully-busy window. Countermeasures:
  - DMAs issue first (weights then two big xt transfers on SP); a dense
    FD=512 warm-up outlasts them and chains into the k0/q0 projections.
  - full-width filler matmuls ([128,128] ZERO stationary x [128,512],
    i.e. exact no-op zero-adds into the armed partials bank) are
    interleaved between every scores/attn pack; narrow [1,1] fillers do
    NOT register with the activity monitor.  ~2 fillers/quad both keeps
    every HAM window busy and soaks up pipeline jitter; when a gap does
    slip through, the dense cadence re-warms K within a window or two.
  - Tile reorders the PE queue by dependences, so placement is enforced
    with data deps (the partials WAW chain; bridge fillers read qTs so
    they land exactly at the prologue hand-off).

Softmax w/o max subtraction is safe: scores range ~[-50, 54];
exp(54) ~ 2e23 << fp32/bf16 max; row sums < 1e27.
"""

import numpy as np

import concourse.bass as bass
import concourse.tile as tile
from concourse import bacc, mybir
from concourse.bass_utils import run_bass_kernel_spmd
from concourse.masks import make_identity

F32 = mybir.dt.float32
BF16 = mybir.dt.bfloat16

B = 8
L = 4096          # tokens per image (64*64)
C = 192           # channels
D = 24            # head dim (q/k/v)
G = 4             # PE array packing groups
NCH = L // 128    # 32 chunks of 128 tokens
W = 512           # i-window (moving free dim per matmul)
NIW = L // W      # 8 i-windows
WIN = 512         # projection window (rhs free dim)
NWIN = L // WIN   # 8 windows
NQ = NCH // G     # 8 quads of key chunks
VW = 32           # padded vhat chunk width (v | ones | zeros)

N_WARMUP = 20            # FD=512 dense warm-up matmuls (covers HAM ramp + DMAs)


def filler_schedule(iw, t):
    """(pre_scores, pre_attn) filler counts per quad.

    Steady state: 1+1 (~0.43us of full-width PE streaming per ~1.3us quad
    keeps every HAM window busy).  Window 0 pipeline-fills with long exp
    waits, but most of that slack is absorbed by the chased projection
    pairs; a small extra burst covers the rest.
    """
    if iw == 0:
        return {0: (0, 2), 1: (1, 2), 2: (1, 2), 3: (1, 2)}.get(t, (1, 1))
    return (1, 1)


def build_graph():
    """Build the single-core Bass graph (SPMD: identical on all 8 cores)."""
    nc = bacc.Bacc(
        "TRN2", target_bir_lowering=False, debug=False, num_devices=8,
        name="attn_dp",
    )

    x_ext = nc.dram_tensor("x", [L, C], F32, kind="ExternalInput").ap()
    xt_ext = nc.dram_tensor("xt", [C, L], BF16, kind="ExternalInput").ap()
    # group-stacked projection weights [C, 128]: col 32g+d = w[:, d]
    wqs_ext = nc.dram_tensor("wq_stack", [C, 128], F32,
                             kind="ExternalInput").ap()
    wks_ext = nc.dram_tensor("wk_stack", [C, 128], F32,
                             kind="ExternalInput").ap()
    # wo_stack [128, 193]: rows 32g+d = gamma * w_o[d]; rows 32g+24 col 192 = 1
    wos_ext = nc.dram_tensor("wo_stack", [128, C + 1], F32,
                             kind="ExternalInput").ap()
    out_ext = nc.dram_tensor("out", [L, C], F32, kind="ExternalOutput").ap()

    with tile.TileContext(nc) as tc:
        _build(tc, x_ext, xt_ext, wqs_ext, wks_ext, wos_ext, out_ext)

    nc.compile()
    return nc


def _build(tc, x_ext, xt_ext, wqs_ext, wks_ext, wos_ext, out_ext):
    nc = tc.nc

    with (
        # ---- persistent SBUF ----
        tc.tile_pool(name="const", bufs=1) as const_pool,
        tc.tile_pool(name="xsb", bufs=1) as x_pool,
        tc.tile_pool(name="xT", bufs=1) as xT_pool,
        tc.tile_pool(name="qkT", bufs=1) as qkT_pool,
        tc.tile_pool(name="vhat", bufs=1) as vhat_pool,
        tc.tile_pool(name="expS", bufs=6) as expS_pool,
        tc.tile_pool(name="pt", bufs=3) as pt_pool,
        tc.tile_pool(name="outst", bufs=3) as outst_pool,
        tc.tile_pool(name="rden", bufs=6) as r_pool,
        # ---- PSUM (8 banks): scores/proj/epilogue 3x2 + partials 2x1 ----
        tc.tile_pool(name="ps_s", bufs=3, space="PSUM") as ps_scores,
        tc.tile_pool(name="ps_acc", bufs=2, space="PSUM") as ps_partials,
    ):
        # ================= DMAs first (overlap the PE warm-up) ==========
        # two big xt DMAs on the sync (SP) ring; weights on the scalar
        # (Act) ring: both rings issue in parallel, everything lands well
        # inside the warm-up.
        wstage = const_pool.tile([128, 760], F32)
        SR = 128
        st_wqsa = wstage[:, 0:SR]
        st_wqsb = wstage[:64, SR:2 * SR]
        st_wksa = wstage[:, 2 * SR:3 * SR]
        st_wksb = wstage[:64, 3 * SR:4 * SR]
        st_wos = wstage[:, 4 * SR:4 * SR + C + 1]
        nc.sync.dma_start(st_wksa, wks_ext[0:128, :])
        nc.sync.dma_start(st_wksb, wks_ext[128:192, :])
        nc.sync.dma_start(st_wqsa, wqs_ext[0:128, :])
        nc.sync.dma_start(st_wqsb, wqs_ext[128:192, :])
        nc.sync.dma_start(st_wos, wos_ext)

        xTa = xT_pool.tile([128, L], BF16)            # x^T rows 0..127
        xTb = xT_pool.tile([64, L], BF16)             # x^T rows 128..191
        nc.sync.dma_start(xTa[:], xt_ext[0:128, :])
        nc.sync.dma_start(xTb[:], xt_ext[128:192, :])

        # ================= constants (cheap engines) =================
        # warm-up dependencies first: the gpsimd queue runs these serially
        # and the first warm-up matmul gates on them
        ident = const_pool.tile([128, 128], BF16)
        make_identity(nc, ident[:])
        warm_rhs = const_pool.tile([128, W], BF16)
        nc.gpsimd.memset(warm_rhs[:], 0.0)

        # zeros for partials-bank arming + HAM filler matmuls.  The filler
        # stationary must be FULL-WIDTH: the HAM activity monitor does not
        # register narrow ([1,1]-stationary) matmuls as PE activity.
        zl = const_pool.tile([1, 128], BF16)
        zr = const_pool.tile([1, W], BF16)
        zlD = const_pool.tile([128, 128], BF16)
        nc.gpsimd.memset(zl[:], 0.0)
        nc.gpsimd.memset(zr[:], 0.0)
        nc.gpsimd.memset(zlD[:], 0.0)

        ones8 = const_pool.tile([8, L], BF16)
        nc.gpsimd.memset(ones8[:], 0.0)
        nc.gpsimd.memset(ones8[0:1, :], 1.0)

        # preload the exp_and_others ACT table set (~2.7us) during the
        # prologue so the first real exp doesn't stall the loop
        escr = const_pool.tile([1, 2], F32)
        nc.scalar.activation(escr[:], zl[0:1, 0:2],
                             mybir.ActivationFunctionType.Exp)

        # convert weights to bf16 on the Vector queue (the Scalar queue is
        # busy issuing the weight DMAs -- keeping the copies off it means
        # they chase the DMAs immediately)
        wksa = const_pool.tile([128, SR], BF16, tag="wksa")
        wksb = const_pool.tile([64, SR], BF16, tag="wksb")
        wqsa = const_pool.tile([128, SR], BF16, tag="wqsa")
        wqsb = const_pool.tile([64, SR], BF16, tag="wqsb")
        wos = const_pool.tile([128, C + 1], BF16, tag="wos")
        nc.vector.tensor_copy(wksa[:], st_wksa)
        nc.vector.tensor_copy(wksb[:], st_wksb)
        nc.vector.tensor_copy(wqsa[:], st_wqsa)
        nc.vector.tensor_copy(wqsb[:], st_wqsb)
        nc.vector.tensor_copy(wos[:], st_wos)

        # ================= PE warm-up =================
        # dense back-to-back FD=512 matmuls with no deps: un-throttles the
        # HAM clock gate (1.2 -> 2.4 GHz) and outlasts every prologue DMA,
        # so the projections start warm with all deps met.
        warm_ps = ps_scores.tile([128, W], F32, tag="s", name="warm_ps")
        for _ in range(N_WARMUP):
            nc.tensor.matmul(warm_ps[:], ident[:], warm_rhs[:],
                             start=True, stop=True)

        # ================= persistent tiles =================
        x_sb = x_pool.tile([128, NCH * C], F32)       # chunk c at cols [C*c,)
        kTs = qkT_pool.tile([128, L], BF16)           # stacked k^T replicas
        qTs = qkT_pool.tile([128, L], BF16)           # stacked q^T replicas
        vhat = vhat_pool.tile([128, NCH * VW], BF16)  # v | ones | zero pad
        vt = nc.dram_tensor("vt_scratch", [VW, L], BF16).ap()

        # ======== chased projections ========
        # window pair p: one [128,1024] PSUM tile = proj(2p) | proj(2p+1);
        # one 1024-col copy out, alternating ScalarE / VectorE.  Only pair 0
        # of k and q (plus vhat pair 0) runs in the prologue; the remaining
        # pairs are emitted inside the early windows, just ahead of the
        # quads that consume them -- real work that replaces fill-phase
        # fillers and cuts ~10us off the prologue span.
        def project_pair(dst, wa, wb, p, nm):
            ps = ps_scores.tile([128, 1024], F32, tag="s", name=f"pj{nm}{p}")
            for h in range(2):
                w = 2 * p + h
                sl = slice(WIN * w, WIN * (w + 1))
                psl = slice(512 * h, 512 * (h + 1))
                nc.tensor.matmul(ps[:, psl], wa[:], xTa[:, sl],
                                 start=True, stop=False)
                nc.tensor.matmul(ps[:, psl], wb[:], xTb[:, sl],
                                 start=False, stop=True)
            dsl = slice(1024 * p, 1024 * (p + 1))
            par = p if nm == "k" else p + 1   # k0 on Sc, q0 on Ve, ...
            if par % 2 == 0:
                nc.scalar.copy(dst[:, dsl], ps[:])
            else:
                nc.vector.tensor_copy(dst[:, dsl], ps[:])

        vhat_view = vhat.rearrange("p (j d) -> p j d", d=VW)

        def emit_vt_pair(p):
            # v^T strips for token pair p out of kTs -> DRAM, then
            # xbar-transpose back as token-major vhat chunks 8p..8p+7.
            # Odd pairs ride the otherwise-idle scalar (Act) ring so the
            # four round trips don't serialize behind x/out traffic on SP.
            eng = nc.scalar if p % 2 else nc.sync
            csl = slice(1024 * p, 1024 * (p + 1))
            eng.dma_start(vt[0:8, csl], kTs[24:32, csl])
            eng.dma_start(vt[8:16, csl], kTs[56:64, csl])
            eng.dma_start(vt[16:24, csl], kTs[88:96, csl])
            eng.dma_start(vt[24:32, csl], ones8[:, csl])
            eng.dma_start_transpose(out=vhat_view[:, 8 * p:8 * (p + 1), :],
                                    in_=vt[:, csl])

        project_pair(kTs, wksa, wksb, 0, "k")
        emit_vt_pair(0)
        project_pair(qTs, wqsa, wqsb, 0, "q")

        pt_tiles = {}
        ot_tiles = {}
        # chunk-major DRAM views: rows (s p) -> [p, s, c] so one DMA moves a
        # whole window's 4 chunks between DRAM and the chunked SBUF layout
        oview = out_ext.rearrange("(s p) c -> p s c", p=128)
        xview = x_ext.rearrange("(s p) c -> p s c", p=128)
        x_sbv = x_sb.rearrange("p (s c) -> p s c", c=C)

        def emit_epilogue(piw, s):
            ptb = pt_tiles[piw]
            cidx = (W // 128) * piw + s  # 128-token chunk index
            if s == 0:
                ot_tiles[piw] = outst_pool.tile([128, 4 * C], F32,
                                                name=f"ot{piw}", tag="ot")
            ot = ot_tiles[piw]
            ep = ps_scores.tile([128, C + 1], F32, tag="s", name=f"ep{piw}_{s}")
            nc.tensor.matmul(ep[:], ptb[:, 128 * s:128 * (s + 1)],
                             wos[:], start=True, stop=True)
            rr = r_pool.tile([128, 1], F32, name=f"rr{piw}_{s}", tag="rr")
            nc.vector.reciprocal(rr[:], ep[:, C:C + 1])
            # out = (un * (1/denom)) + x in one fused DVE op
            nc.vector.scalar_tensor_tensor(
                ot[:, C * s:C * (s + 1)], ep[:, 0:C], rr[:],
                x_sb[:, C * cidx:C * (cidx + 1)],
                op0=mybir.AluOpType.mult, op1=mybir.AluOpType.add)
            if s == W // 128 - 1:
                otv = ot.rearrange("p (s c) -> p s c", c=C)
                nc.sync.dma_start(oview[:, 4 * piw:4 * piw + 4, :], otv)

        # ================= main loop =================
        def emit_attn(partials_, t, ess):
            for g in (2, 3, 0, 1):
                j = G * t + g
                nc.tensor.matmul(
                    partials_[32 * g:32 * g + VW, :],
                    vhat[:, VW * j:VW * (j + 1)],
                    ess[g // 2][:, 512 * (g % 2):512 * (g % 2 + 1)],
                    start=False, stop=(t == NQ - 1),
                    tile_position=(0, 32 * g),
                    skip_group_check=True,
                )

        def emit_fillers(partials_, n, rhs=None):
            # full-width [128,128]x[128,512] zero-add into the armed partials
            # bank: numerically exact no-op (zero stationary), ~215ns of dense
            # PE streaming each that the HAM activity monitor registers.  An
            # explicit rhs gives the filler a data dependency so the
            # scheduler places it at a specific stall point.
            for _ in range(n):
                nc.tensor.matmul(partials_[:, :], zlD[:],
                                 warm_rhs[:] if rhs is None else rhs,
                                 start=False, stop=False,
                                 skip_group_check=True)

        pending_attn = None
        for iw in range(NIW):
            isl = slice(W * iw, W * (iw + 1))
            # residual x chunks for this window's epilogue (runs during iw+2)
            nc.sync.dma_start(x_sbv[:, 4 * iw:4 * iw + 4, :],
                              xview[:, 4 * iw:4 * iw + 4, :])
            partials = ps_partials.tile([128, W], F32, name=f"partials{iw}",
                                        tag="acc")
            # zero-init the bank and set has_written on all 128 partitions so
            # the col-tiled accumulating matmuls below can all use start=False
            nc.tensor.matmul(partials[:, :], zl[:], zr[:],
                             start=True, stop=False, skip_group_check=True)
            if iw == 0:
                # bridge fillers gated on the q-pair0 copy: they occupy the
                # PE exactly across the prologue -> main-loop hand-off
                emit_fillers(partials, 2, rhs=qTs[:, 0:W])
            for t in range(NQ):
                qidx = NQ * iw + t
                n_pre, n_post = filler_schedule(iw, t)
                emit_fillers(partials, n_pre)
                scs = [ps_scores.tile([128, 1024], F32, tag="s",
                                      name=f"sc{iw}_{t}_0"),
                       ps_scores.tile([128, 1024], F32, tag="s",
                                      name=f"sc{iw}_{t}_1")]
                for g in range(G):
                    j = G * t + g
                    nc.tensor.matmul(
                        scs[g // 2][:, 512 * (g % 2):512 * (g % 2 + 1)],
                        kTs[32 * g:32 * g + 32, 128 * j:128 * (j + 1)],
                        qTs[32 * g:32 * g + 32, isl],
                        start=True, stop=True,
                        tile_position=(32 * g, 0),
                    )
                # chase the remaining projection pairs just ahead of their
                # consumers: k-pair/vhat-pair p before scores quad t=2p of
                # window 0; q-pair p two windows ahead of window 2p
                if iw == 0 and t in (0, 1, 2):
                    project_pair(kTs, wksa, wksb, t + 1, "k")
                    emit_vt_pair(t + 1)
                if (iw, t) in ((0, 6), (2, 0), (4, 0)):
                    project_pair(qTs, wqsa, wqsb, iw // 2 + 1, "q")
                # exp split: ScalarE takes h=0; DVE takes h=1, except every
                # 8th quad splits its h=1 tile across BOTH engines (net
                # ratio ~72/56 balances engine load without a 2-tile Sc
                # hiccup feeding queue jitter)
                split_q = (qidx % 8) == 0
                ess = []
                for h in range(2):
                    es = expS_pool.tile([128, 1024], BF16,
                                        name=f"es{iw}_{t}_{h}", tag="es")
                    if h == 0:
                        nc.scalar.activation(es[:], scs[h][:],
                                             mybir.ActivationFunctionType.Exp)
                    elif split_q:
                        nc.scalar.activation(es[:, 0:512], scs[h][:, 0:512],
                                             mybir.ActivationFunctionType.Exp)
                        nc.vector.tensor_scalar(
                            es[:, 512:1024].bitcast(mybir.dt.int16),
                            scs[h][:, 512:1024],
                            184.66496580927026, 16256.0,
                            op0=mybir.AluOpType.mult, op1=mybir.AluOpType.add)
                    else:
                        # Schraudolph exp on DVE: bf16 bits = round(s*log2e*128
                        # + 127*128); one fused mult+add with int16 convert
                        nc.vector.tensor_scalar(
                            es[:].bitcast(mybir.dt.int16), scs[h][:],
                            184.66496580927026, 16256.0,
                            op0=mybir.AluOpType.mult, op1=mybir.AluOpType.add)
                    ess.append(es)
                # epilogue deferred TWO windows: the ptb copy then has a full
                # window to drain through the exp backlog, so the in-order
                # PE queue never stalls on the epilogue matmul's ptb wait
                if iw >= 2 and t < W // 128:
                    emit_epilogue(iw - 2, t)
                if iw == NIW - 1 and t >= NQ - W // 128:
                    # squeeze the penultimate window's epilogues into the
                    # back half of the last window to shorten the tail
                    emit_epilogue(NIW - 2, t - (NQ - W // 128))
                emit_fillers(partials, n_post)
                # attention emission is deferred one quad: the PE fills the
                # exp wait with the next quad's scores instead of idling
                if pending_attn is not None:
                    emit_attn(*pending_attn)
                pending_attn = (partials, t, ess)

            emit_attn(*pending_attn)
            pending_attn = None
            ptb = pt_pool.tile([128, W], BF16, name=f"ptb{iw}", tag="ptb")
            nc.scalar.copy(ptb[:], partials[:])
            pt_tiles[iw] = ptb
        for piw in (NIW - 2, NIW - 1):
            for s in range(W // 128):
                emit_epilogue(piw, s)


_CACHE = {}


def _get_graph():
    if "nc" not in _CACHE:
        _CACHE["nc"] = build_graph()
    return _CACHE["nc"]


def make_in_maps(tensor, w_q, w_k, w_v, w_o, gamma):
    import ml_dtypes
    x = np.ascontiguousarray(np.asarray(tensor, dtype=np.float32)).reshape(B, L, C)
    xt = np.ascontiguousarray(
        x.transpose(0, 2, 1).astype(ml_dtypes.bfloat16))  # [B, C, L] bf16
    wq = np.asarray(w_q, dtype=np.float32)
    wk = np.asarray(w_k, dtype=np.float32)
    wv = np.ascontiguousarray(np.asarray(w_v, dtype=np.float32))
    wo = np.asarray(w_o, dtype=np.float32)

    wq_stack = np.zeros((C, 128), dtype=np.float32)
    wk_stack = np.zeros((C, 128), dtype=np.float32)
    for g in range(G):
        wq_stack[:, 32 * g:32 * g + D] = wq
        wk_stack[:, 32 * g:32 * g + D] = wk
    # w_v rides in the spare zero columns of wk_stack (contracts against
    # zero rows of the q stack, so scores are unaffected); the k-projection
    # then produces v^T rows for free.
    wk_stack[:, 24:32] = wv[:, 0:8]
    wk_stack[:, 56:64] = wv[:, 8:16]
    wk_stack[:, 88:96] = wv[:, 16:24]

    wo_stack = np.zeros((128, C + 1), dtype=np.float32)
    for g in range(G):
        wo_stack[32 * g:32 * g + D, :C] = wo * np.float32(gamma)
        wo_stack[32 * g + D, C] = 1.0

    return [
        {"x": np.ascontiguousarray(x[b]), "xt": xt[b], "wq_stack": wq_stack,
         "wk_stack": wk_stack, "wo_stack": wo_stack}
        for b in range(B)
    ]


def kernel(tensor, w_q, w_k, w_v, w_o, gamma):
    nc = _get_graph()
    in_maps = make_in_maps(tensor, w_q, w_k, w_v, w_o, gamma)
    res = run_bass_kernel_spmd(nc, in_maps, core_ids=list(range(B)))
    out = np.stack([np.asarray(res.results[b]["out"]) for b in range(B)])
    return out.reshape(B, 64, 64, C).astype(np.float32)
